# revision 30
# baseline (speedup 1.0000x reference)
"""MoE feed-forward (E=8 experts, top-2) for one TRN2 chip (8 NeuronCores).

Strategy: expert-parallel. Host computes the (tiny) router matmul + softmax
+ top-2 in numpy, gathers each expert's routed tokens, pads to a fixed
capacity C (~1078 for the expected routing), and ships per-expert weights
+ gathered tokens to one core each.  Each core runs an identical Bass/Tile
FFN program in bf16:

    GT = Wg^T @ X   (transposed-activation layout: [I, C] tiles)
    UT = Wu^T @ X
    AT = silu(GT) * UT          (bf16, SBUF-resident)
    YT = Wd^T_col-tiles @ AT    -> [H, C] bf16 out

The PE stream (~173us of bf16 matmul) is the wall; everything else is kept
off the critical path:
  - token chunks are (360, 512, C-872): chunk 0 stays 360 wide (the
    startup junk taper is tuned to its ~90KB x-piece DMA cadence), and the
    LAST chunk is small so the final cast + output-DMA tail after the very
    last matmul is short;
  - phase A runs chunk-major, phase B (down-proj) h-tile passes interleave
    (A0, A1, B0, A2, B1, B2) so the down-projection never waits on the
    tail of phase A and output DMA spreads over the program;
  - input DMAs are issued from both HWDGE queues (SP + ACT) in consumption
    order; ACT carries ONLY i-tile 0's weights (wg0 split per-c so the
    first matmul waits on 32KB) — a deeper ACT queue would block the
    scalar engine past its ring depth and delay the first silu, backing up
    all 8 PSUM banks and stalling the PE;
  - the PE clock ramps 0.65->1.2->2.4GHz and needs ~3us of GAPLESS
    execution for full speed (a stall resets the ramp), so dependency-free
    junk matmuls bridge the fixed ~6.5us engine-start latency and the
    DMA-paced first sweep;
  - a dummy 1-column silu is placed right AFTER the scalar queue's DMA
    issues: it pulls the ~1.3us Silu ACT_TABLE_LOAD into the warmup era
    without delaying the wg0-piece DMAs the first matmuls wait on;
  - y is written out in bf16 (error contribution ~0.2% of an output that
    has ~8x that from the bf16 matmuls).

The host applies the top-2 combine weights and scatters rows back into the
full [B, S, H] output.  End-to-end rel err ~4.4e-3 (budget 2e-2).
"""

import numpy as np
import ml_dtypes

H = 1024
I = 2048
E = 8
TOPK = 2
P = 128

_PROGRAM_CACHE = {}
LAST_RESULT = None  # BassKernelResults of the most recent device run


def _build_program(C):
    from contextlib import ExitStack

    import concourse.mybir as mybir
    import concourse.tile as tile
    from concourse import bacc

    f32 = mybir.dt.float32
    bf16 = mybir.dt.bfloat16
    Silu = mybir.ActivationFunctionType.Silu

    n_h = H // P   # 8 contraction chunks over hidden dim
    n_i = I // P   # 16 tiles over intermediate dim
    # Asymmetric token chunks: chunk 0 stays 360 wide (the startup taper
    # is tuned to its ~90KB x-piece cadence), chunk 1 takes a full PSUM
    # bank (512), and the remainder lands in a small LAST chunk so the
    # final cast + output-DMA tail after the last matmul is short.
    k1 = min(512, C - 360 - 128)
    widths = [360, k1, C - 360 - k1]
    assert all(82 <= w <= 512 for w in widths), widths
    starts = [0, 360, 360 + k1]
    NTmax = max(widths)

    nc = bacc.Bacc("TRN2", enable_partition_id=False)
    xT = nc.dram_tensor("xT", [H, C], bf16, kind="ExternalInput")
    wg = nc.dram_tensor("wg", [P, I // P, H // P, P], bf16, kind="ExternalInput")
    wu = nc.dram_tensor("wu", [P, I // P, H // P, P], bf16, kind="ExternalInput")
    wd = nc.dram_tensor("wd", [I, H], bf16, kind="ExternalInput")
    yT = nc.dram_tensor("yT", [H, C], bf16, kind="ExternalOutput")

    with tile.TileContext(nc) as tc:
        with ExitStack() as ctx:
            wpool = ctx.enter_context(tc.tile_pool(name="weights", bufs=1))
            atpool = ctx.enter_context(tc.tile_pool(name="atp", bufs=1))
            spool = ctx.enter_context(tc.tile_pool(name="stmp", bufs=4))
            ypool = ctx.enter_context(tc.tile_pool(name="yst", bufs=4))
            pspool = ctx.enter_context(
                tc.tile_pool(name="ps", bufs=8, space="PSUM")
            )

            warm_src = wpool.tile([P, P], bf16, name="warm_src")
            nc.vector.memset(warm_src, 0.0)
            warm_ps = pspool.tile([P, NTmax], f32, tag="ps", name="warm_ps")

            def junk_mms(n):
                for _ in range(n):
                    nc.tensor.matmul(
                        warm_ps[:, 0:P], warm_src, warm_src,
                        start=True, stop=True,
                    )

            junk_mms(26)

            x_s = wpool.tile([P, n_h, C], bf16, name="x_s")
            wg_s = wpool.tile([P, n_i, n_h, P], bf16, name="wg_s")
            wu_s = wpool.tile([P, n_i, n_h, P], bf16, name="wu_s")
            wd_s = wpool.tile([P, n_i, H], bf16, name="wd_s")
            at_s = atpool.tile([P, n_i, C], bf16, name="at_s")

            k0w = widths[0]
            for c in range(n_h):
                nc.scalar.dma_start(
                    out=wg_s[:, 0, c:c + 1, :], in_=wg[:, 0, c:c + 1, :]
                )
            nc.scalar.dma_start(out=wu_s[:, 0, :, :], in_=wu[:, 0, :, :])
            for c in range(n_h):
                nc.sync.dma_start(
                    out=x_s[:, c, 0:k0w], in_=xT[c * P:(c + 1) * P, 0:k0w]
                )
            nc.sync.dma_start(out=wg_s[:, 1, :, :], in_=wg[:, 1, :, :])
            nc.sync.dma_start(out=wu_s[:, 1, :, :], in_=wu[:, 1, :, :])
            for it in range(2, n_i):
                nc.sync.dma_start(out=wg_s[:, it, :, :], in_=wg[:, it, :, :])
                nc.sync.dma_start(out=wu_s[:, it, :, :], in_=wu[:, it, :, :])
            for c in range(n_h):
                nc.sync.dma_start(
                    out=x_s[:, c, k0w:C], in_=xT[c * P:(c + 1) * P, k0w:C]
                )
            for it in range(n_i):
                nc.sync.dma_start(
                    out=wd_s[:, it, :], in_=wd[it * P:(it + 1) * P, :]
                )

            # Dummy activation AFTER the scalar queue's DMA issues: pulls
            # the ~1.3us Silu ACT_TABLE_LOAD into the DMA-paced warmup era
            # (so the first real silu doesn't stall PSUM) without delaying
            # the wg0-piece DMAs the first matmuls wait on.
            warm_act = spool.tile([P, 1], f32, tag="stmp", name="warm_act")
            nc.scalar.activation(warm_act, warm_src[:, 0:1], Silu)

            def a_pass(k):
                c0, w = starts[k], widths[k]
                for it in range(n_i):
                    g_ps = pspool.tile([P, w], f32, tag="ps", name=f"g_{it}_{k}")
                    u_ps = pspool.tile([P, w], f32, tag="ps", name=f"u_{it}_{k}")
                    for c in range(n_h):
                        st, sp = (c == 0), (c == n_h - 1)
                        nc.tensor.matmul(
                            g_ps, wg_s[:, it, c, :],
                            x_s[:, c, c0:c0 + w], start=st, stop=sp,
                        )
                        if k == 0 and it == 0 and c < n_h - 1:
                            junk_mms((10, 10, 6, 3, 2, 1, 0)[c])
                    for c in range(n_h):
                        st, sp = (c == 0), (c == n_h - 1)
                        nc.tensor.matmul(
                            u_ps, wu_s[:, it, c, :],
                            x_s[:, c, c0:c0 + w], start=st, stop=sp,
                        )
                    stile = spool.tile([P, w], f32, tag="stmp", name=f"s_{it}_{k}")
                    nc.scalar.activation(stile, g_ps, Silu)
                    nc.vector.tensor_mul(
                        at_s[:, it, c0:c0 + w], stile, u_ps
                    )

            def b_pass(k, last=False):
                c0, w = starts[k], widths[k]
                for ht in range(n_h):
                    y_ps = pspool.tile([P, w], f32, tag="ps", name=f"y_{ht}_{k}")
                    for it in range(n_i):
                        st, sp = (it == 0), (it == n_i - 1)
                        nc.tensor.matmul(
                            y_ps, wd_s[:, it, ht * P:(ht + 1) * P],
                            at_s[:, it, c0:c0 + w], start=st, stop=sp,
                        )
                    yt = ypool.tile([P, w], bf16, tag="yst", name=f"yo_{ht}_{k}")
                    nc.vector.tensor_copy(yt, y_ps)
                    if not (last and ht == n_h - 1):
                        nc.sync.dma_start(
                            out=yT[ht * P:(ht + 1) * P, c0:c0 + w],
                            in_=yt,
                        )
                    else:
                        q = P // 4
                        for r in range(4):
                            eng = nc.sync if r % 2 == 0 else nc.scalar
                            eng.dma_start(
                                out=yT[ht * P + r * q:ht * P + (r + 1) * q,
                                       c0:c0 + w],
                                in_=yt[r * q:(r + 1) * q, :],
                            )

            a_pass(0)
            a_pass(1)
            b_pass(0)
            a_pass(2)
            b_pass(1)
            b_pass(2, last=True)

    nc.compile()
    return nc


def kernel(x, gate_w, wg, wu, wd):
    global LAST_RESULT
    x = np.asarray(x, dtype=np.float32)
    gate_w = np.asarray(gate_w, dtype=np.float32)
    wg = np.asarray(wg, dtype=np.float32)
    wu = np.asarray(wu, dtype=np.float32)
    wd = np.asarray(wd, dtype=np.float32)

    B, S, Hh = x.shape
    T = B * S
    xf = np.ascontiguousarray(x.reshape(T, Hh))

    logits = xf @ gate_w.T
    logits -= logits.max(axis=-1, keepdims=True)
    np.exp(logits, out=logits)
    probs = logits / logits.sum(axis=-1, keepdims=True)
    order = np.argsort(-probs, axis=1, kind="stable")[:, :TOPK]

    onehot = np.zeros((T, E), dtype=bool)
    onehot[np.arange(T)[:, None], order] = True
    tok_lists = [np.nonzero(onehot[:, e])[0] for e in range(E)]
    maxc = max(max(len(t) for t in tok_lists), 600)
    C = maxc + (maxc % 2)  # round up to even
    assert C <= 1344, f"expert load too imbalanced for this kernel: {maxc}"

    nc = _PROGRAM_CACHE.get(C)
    if nc is None:
        nc = _build_program(C)
        _PROGRAM_CACHE[C] = nc

    bf = ml_dtypes.bfloat16
    xf_bf = xf.astype(bf)

    def _gu_layout(w):
        return np.ascontiguousarray(
            w.reshape(H // P, P, I // P, P).transpose(1, 2, 0, 3)
        )

    in_maps = []
    for e in range(E):
        idx = tok_lists[e]
        xe = np.zeros((C, Hh), dtype=bf)
        xe[: len(idx)] = xf_bf[idx]
        in_maps.append(
            {
                "xT": np.ascontiguousarray(xe.T),
                "wg": _gu_layout(wg[e].astype(bf)),
                "wu": _gu_layout(wu[e].astype(bf)),
                "wd": wd[e].astype(bf),
            }
        )

    from concourse.bass_utils import run_bass_kernel_spmd

    res = run_bass_kernel_spmd(nc, in_maps, core_ids=list(range(E)))
    LAST_RESULT = res

    out = np.zeros((T, Hh), dtype=np.float32)
    for e in range(E):
        idx = tok_lists[e]
        ye = np.asarray(res.results[e]["yT"]).T[: len(idx)]
        out[idx] += probs[idx, e][:, None] * ye.astype(np.float32)
    return out.reshape(B, S, Hh)
